# revision 3
# baseline (speedup 1.0000x reference)
"""Trainium2 Bass kernel for nn_BitwiseLinear.

Reference semantics (B=32768, IN=OUT=1024):
    out = in_scale * weight_scale * (sign(x) @ sign(weight * gate_mask).T + bias)
    gate_mask = (sign(gate)+1)/2; in_scale = mean|x| per row; weight_scale = mean|w| per out.

Notes on the math used here:
    sign(weight * gate_mask) == sign(weight) * (gate >= 0)    (gate==0 -> mask 0.5 -> sign(w))
    out = (sum|x|_row) * (psum + bias_eff_term) where the weight operand has
    weight_scale * 2^-20 folded in (2^-20 = 1/1024^2, the two mean divisors).

Sharding: data-parallel over batch across 8 cores; weight/gate/bias replicated.
Each core: x shard [4096, 1024] -> out shard [4096, 1024].
"""

import numpy as np

import concourse.bacc as bacc
import concourse.mybir as mybir
import concourse.tile as tile
from concourse import masks
from concourse.bass_utils import run_bass_kernel_spmd

B, IN, OUT = 32768, 1024, 1024
NCORES = 8
BSH = B // NCORES            # 4096 rows per core
P = 128                      # partitions
NT = BSH // P                # 32 x-tiles per core
KC = IN // P                 # 8 contraction chunks of 128
NCH = 512                    # matmul moving free-dim (one PSUM bank of f32)
F32 = mybir.dt.float32
BF16 = mybir.dt.bfloat16
WS_SCALE = float(2.0 ** -20)  # 1/(1024*1024): folds both mean divisors

_CACHE: dict = {}


def _build():
    nc = bacc.Bacc("TRN2", target_bir_lowering=False, debug=False,
                   num_devices=NCORES)

    x_ext = nc.declare_dram_parameter("x", [BSH, IN], F32, isOutput=False)
    w_ext = nc.declare_dram_parameter("weight", [OUT, IN], F32, isOutput=False)
    g_ext = nc.declare_dram_parameter("gate", [OUT, IN], F32, isOutput=False)
    b_ext = nc.declare_dram_parameter("bias", [1, OUT], F32, isOutput=False)
    o_ext = nc.declare_dram_parameter("out", [BSH, OUT], F32, isOutput=True)

    x_ap = x_ext.ap()
    w_ap = w_ext.ap()
    g_ap = g_ext.ap()
    b_ap = b_ext.ap()
    o_ap = o_ext.ap()

    ACT = mybir.ActivationFunctionType
    ALU = mybir.AluOpType
    AX = mybir.AxisListType

    with tile.TileContext(nc) as tc:
        with tc.tile_pool(name="const", bufs=1) as cp:
            ident_f32 = cp.tile([P, P], F32)
            masks.make_identity(nc, ident_f32[:])
            ident_bf = cp.tile([P, P], BF16)
            masks.make_identity(nc, ident_bf[:])
            ones_bf = cp.tile([1, P], BF16)
            nc.gpsimd.memset(ones_bf[:], 1.0)
            ones_f32 = cp.tile([1, P], F32)
            nc.gpsimd.memset(ones_f32[:], 1.0)
            zbias = cp.tile([P, 1], F32)
            nc.gpsimd.memset(zbias[:], 0.0)

            # persistent prepped weights
            wTq = cp.tile([P, KC * OUT], BF16)    # chunk c: [128 i_c, 1024 o] at [:, c*OUT:]
            bias_eff = cp.tile([1, OUT], BF16)    # bias * ws * 2^-20
            ws_bcast = cp.tile([P, OUT], F32)     # ws * 2^-20 broadcast over partitions

            # ---------------- weight prep ----------------
            with tc.tile_pool(name="wprep", bufs=2) as wp, \
                 tc.tile_pool(name="wkeep", bufs=1) as wk, \
                 tc.tile_pool(name="wpsum1", bufs=1, space="PSUM") as wps1, \
                 tc.tile_pool(name="wpsum", bufs=2, space="PSUM") as wps:
                w_bin = wk.tile([P, KC * IN], BF16)   # o-tile t: [128 o_t, 1024 i] at [:, t*IN:]
                ws_cols = wk.tile([P, KC], F32)       # per-o |w| row sums, o-tile t in col t
                bias_sb = wk.tile([1, OUT], F32)
                ws_row = wk.tile([1, OUT], F32)
                nc.sync.dma_start(bias_sb[:], b_ap[:, :])

                for t in range(KC):
                    wt = wp.tile([P, IN], F32)
                    nc.sync.dma_start(wt[:], w_ap[t * P:(t + 1) * P, :])
                    gt = wp.tile([P, IN], F32)
                    nc.sync.dma_start(gt[:], g_ap[t * P:(t + 1) * P, :])
                    nc.vector.tensor_reduce(ws_cols[:, t:t + 1], wt[:], axis=AX.X,
                                            op=ALU.add, apply_absolute_value=True)
                    sgn = wp.tile([P, IN], BF16)
                    nc.scalar.activation(sgn[:], wt[:], ACT.Sign, bias=zbias[:])
                    msk = wp.tile([P, IN], BF16)
                    nc.vector.tensor_scalar(msk[:], gt[:], 0.0, None, op0=ALU.is_ge)
                    nc.vector.tensor_tensor(w_bin[:, t * IN:(t + 1) * IN], sgn[:],
                                            msk[:], op=ALU.mult)

                # ws_row[0, o] = sum_i |w[o, i]|, assembled via 8 tiny PE transposes
                ps_row = wps1.tile([1, OUT], F32)
                for t in range(KC):
                    nc.tensor.transpose(ps_row[0:1, t * P:(t + 1) * P],
                                        ws_cols[:, t:t + 1], ident_f32[:])
                nc.scalar.activation(ws_row[:], ps_row[:], ACT.Copy, scale=WS_SCALE)

                nc.vector.tensor_tensor(bias_eff[:], bias_sb[:], ws_row[:], op=ALU.mult)

                # broadcast ws_row across partitions with a K=1 matmul
                ps_bc = wps1.tile([P, OUT], F32, tag="ps_row")
                for n in range(OUT // NCH):
                    nc.tensor.matmul(ps_bc[:, n * NCH:(n + 1) * NCH], ones_f32[:],
                                     ws_row[:, n * NCH:(n + 1) * NCH])
                nc.vector.tensor_copy(ws_bcast[:], ps_bc[:])

                # wTq chunk c = transpose(w_bin)[i_c, :] * ws_bcast
                for c in range(KC):
                    ps_wt = wps.tile([P, OUT], BF16, tag="ps_wt")
                    for t in range(KC):
                        nc.tensor.transpose(
                            ps_wt[:, t * P:(t + 1) * P],
                            w_bin[:, t * IN + c * P: t * IN + (c + 1) * P],
                            ident_bf[:])
                    nc.vector.tensor_tensor(wTq[:, c * OUT:(c + 1) * OUT], ps_wt[:],
                                            ws_bcast[:], op=ALU.mult)

            # ---------------- main loop over x tiles ----------------
            with tc.tile_pool(name="xin", bufs=3) as xin_pool, \
                 tc.tile_pool(name="xbt", bufs=2) as xbt_pool, \
                 tc.tile_pool(name="osb", bufs=3) as osb_pool, \
                 tc.tile_pool(name="sc", bufs=4) as sc_pool, \
                 tc.tile_pool(name="pst", bufs=2, space="PSUM") as pst_pool, \
                 tc.tile_pool(name="pso", bufs=2, space="PSUM") as pso_pool:
                for it in range(NT):
                    xt = xin_pool.tile([P, IN], F32)
                    nc.sync.dma_start(xt[:], x_ap[it * P:(it + 1) * P, :])

                    is_raw = sc_pool.tile([P, 1], F32)
                    nc.vector.tensor_reduce(is_raw[:], xt[:], axis=AX.X,
                                            op=ALU.add, apply_absolute_value=True)

                    ps_t = pst_pool.tile([P, IN], F32)
                    for c in range(KC):
                        nc.tensor.transpose(ps_t[:, c * P:(c + 1) * P],
                                            xt[:, c * P:(c + 1) * P], ident_f32[:])

                    xbT = xbt_pool.tile([P, IN], BF16)
                    nc.scalar.activation(xbT[:], ps_t[:], ACT.Sign, bias=zbias[:])

                    ps_o = pso_pool.tile([P, OUT], F32)
                    for c in range(KC):
                        for n in range(OUT // NCH):
                            nc.tensor.matmul(
                                ps_o[:, n * NCH:(n + 1) * NCH],
                                xbT[:, c * P:(c + 1) * P],
                                wTq[:, c * OUT + n * NCH: c * OUT + (n + 1) * NCH],
                                start=(c == 0), stop=False)
                    for n in range(OUT // NCH):
                        nc.tensor.matmul(ps_o[:, n * NCH:(n + 1) * NCH], ones_bf[:],
                                         bias_eff[:, n * NCH:(n + 1) * NCH],
                                         start=False, stop=True)

                    out_sb = osb_pool.tile([P, OUT], F32)
                    nc.scalar.activation(out_sb[:, 0:NCH], ps_o[:, 0:NCH], ACT.Copy,
                                         scale=is_raw[:])
                    nc.vector.tensor_scalar(out_sb[:, NCH:OUT], ps_o[:, NCH:OUT],
                                            is_raw[:], None, op0=ALU.mult)

                    nc.sync.dma_start(o_ap[it * P:(it + 1) * P, :], out_sb[:])

    nc.compile()
    return nc


def _get_nc():
    if "nc" not in _CACHE:
        _CACHE["nc"] = _build()
    return _CACHE["nc"]


def run(x, weight, gate, bias, trace=False):
    nc = _get_nc()
    x = np.ascontiguousarray(np.asarray(x, dtype=np.float32))
    weight = np.ascontiguousarray(np.asarray(weight, dtype=np.float32))
    gate = np.ascontiguousarray(np.asarray(gate, dtype=np.float32))
    bias = np.ascontiguousarray(np.asarray(bias, dtype=np.float32)).reshape(1, OUT)
    in_maps = [
        {"x": x[i * BSH:(i + 1) * BSH], "weight": weight, "gate": gate, "bias": bias}
        for i in range(NCORES)
    ]
    res = run_bass_kernel_spmd(nc, in_maps, core_ids=list(range(NCORES)), trace=trace)
    out = np.concatenate([res.results[i]["out"] for i in range(NCORES)], axis=0)
    return out, res


def kernel(x, weight, gate, bias):
    out, _ = run(x, weight, gate, bias, trace=False)
    return out
